# revision 17
# baseline (speedup 1.0000x reference)
"""MoE layer (8 experts, top-2, SwiGLU) on 8 Trainium2 NeuronCores.

Sharding: expert-parallel. Core m holds expert m's weights (w1/w3/w2) and
computes the full router (replicated small gate) over all T=8192 tokens.
v1 ("dense"): every core runs all tokens through its expert and scales each
token's output by that token's combine weight for this expert (zero when the
expert is not in the token's top-2). The host sums the 8 partial outputs.

The router matmul runs in plain fp32 (top-k decisions need fp32-grade
scores); the expert matmuls optionally run in float32r (full PE rate,
~1.5e-4 rel err) or plain fp32 (1/4 PE rate, ~3e-7).

kernel(**inputs) takes the FULL unsharded inputs and returns the full
(out, aux_loss) like the reference.
"""

import numpy as np

# ---- problem config (hardcoded) ----
B, S, D, H, E, TOPK = 4, 2048, 1024, 4096, 8, 2
T = B * S  # 8192
P = 128

GT = 1024          # token block
SUB = 512          # matmul moving-dim sub-block
NDC = D // P       # 8 d-chunks
NHC = H // P       # 32 h-chunks
NHH = 2            # h-halves (psum can't hold a full block's output)
HCH = NHC // NHH   # 16 h-chunks per half
NBLK = T // GT     # 8 token blocks
NTT = GT // P      # 8 token tiles per block
NSUB = GT // SUB   # 2
NDH = D // SUB     # 2 output d-halves

USE_F32R = True    # expert matmuls in float32r; router stays fp32
USE_SPARSE = True  # route+gather on device, compute only top-2 tokens/expert

CAP = 2304         # per-expert token capacity (max count for seed-0 data: 2182)
CBLKS = [(0, 1024), (1024, 1024), (2048, 256)]  # compact token blocks
NJT = T // P       # 64 router token tiles


def build_program(use_f32r=USE_F32R, use_silu=True):
    import concourse.bacc as bacc
    import concourse.mybir as mybir
    import concourse.tile as tile

    dt = mybir.dt
    f32 = dt.float32
    fmm = dt.float32r if use_f32r else f32
    AF = mybir.ActivationFunctionType
    ALU = mybir.AluOpType
    AX = mybir.AxisListType

    nc = bacc.Bacc("TRN2", target_bir_lowering=False, debug=False, num_devices=E)

    def load(out, in_):
        # f32r tiles need a casting DMA, which only gpsimd issues
        if out.dtype == dt.float32r:
            return nc.gpsimd.dma_start(out=out, in_=in_)
        return nc.sync.dma_start(out=out, in_=in_)

    xT = nc.dram_tensor("xT", [D, T], f32, kind="ExternalInput")
    # gate_aug[:, 0:E] = gate_w ; gate_aug[:, E] = gate_w[:, m] (this core's col)
    gate_aug = nc.dram_tensor("gate_aug", [D, E + 1], f32, kind="ExternalInput")
    # w1p/w3p pre-tiled on host: [NHC, P, D] with [hc, p, dc*P+h] = w[dc*P+p, hc*P+h]
    w1p = nc.dram_tensor("w1p", [NHC, P, D], f32, kind="ExternalInput")
    w3p = nc.dram_tensor("w3p", [NHC, P, D], f32, kind="ExternalInput")
    w2 = nc.dram_tensor("w2", [H, D], f32, kind="ExternalInput")
    y = nc.dram_tensor("y", [T, D], f32, kind="ExternalOutput")
    aux = nc.dram_tensor("aux", [1, 1], f32, kind="ExternalOutput")

    with tile.TileContext(nc) as tc:
        with (
            tc.tile_pool(name="xt", bufs=1) as xt_pool,
            tc.tile_pool(name="xr", bufs=16) as xr_pool,
            tc.tile_pool(name="w13", bufs=2) as w13_pool,
            tc.tile_pool(name="w2p", bufs=HCH + 1) as w2_pool,
            tc.tile_pool(name="g", bufs=HCH + 1) as g_pool,
            tc.tile_pool(name="yac", bufs=NTT * NDH + 1) as yac_pool,
            tc.tile_pool(name="small", bufs=1) as small_pool,
            tc.tile_pool(name="rt", bufs=3) as rt_pool,
            tc.tile_pool(name="cw", bufs=NTT + 2) as cw_pool,
            tc.tile_pool(name="ysb", bufs=3) as y_pool,
            tc.tile_pool(name="gtmp", bufs=2) as gtmp_pool,
            tc.tile_pool(name="ps_s", bufs=2, space="PSUM") as ps_s,
            tc.tile_pool(name="ps_h", bufs=4, space="PSUM") as ps_h,
            tc.tile_pool(name="ps_o", bufs=2, space="PSUM") as ps_o,
        ):
            # ---- constants / persistent ----
            gate_sb = small_pool.tile([P, NDC, E + 1], f32, tag="gate")
            for dc in range(NDC):
                nc.sync.dma_start(
                    out=gate_sb[:, dc, :], in_=gate_aug[dc * P:(dc + 1) * P, :]
                )
            usage = small_pool.tile([P, E], f32, tag="usage")
            nc.vector.memset(usage[:], 0.0)
            ones_col = small_pool.tile([P, 1], f32, tag="ones")
            nc.vector.memset(ones_col[:], 1.0)

            for b in range(NBLK):
                # ---- load x^T block (matmul dtype) ----
                xt = []
                for dc in range(NDC):
                    t_ = xt_pool.tile([P, GT], fmm, tag=f"xt{dc}")
                    load(t_[:], xT[dc * P:(dc + 1) * P, b * GT:(b + 1) * GT])
                    xt.append(t_)

                # ---- router for this block's token tiles (plain fp32) ----
                cw = []
                for tt in range(NTT):
                    ps = ps_s.tile([P, E + 1], f32, tag="ps")
                    for dc in range(NDC):
                        xr = xr_pool.tile([P, P], f32, tag="xr")
                        nc.sync.dma_start(
                            out=xr[:],
                            in_=xT[dc * P:(dc + 1) * P,
                                   b * GT + tt * P: b * GT + (tt + 1) * P],
                        )
                        nc.tensor.matmul(
                            out=ps[:],
                            lhsT=xr[:],
                            rhs=gate_sb[:, dc, :],
                            start=(dc == 0),
                            stop=(dc == NDC - 1),
                        )
                    S_ = rt_pool.tile([P, E + 1], f32, tag="S")
                    nc.vector.tensor_copy(S_[:], ps[:])
                    m8 = rt_pool.tile([P, 8], f32, tag="m8")
                    nc.vector.max(m8[:], S_[:, 0:E])
                    # top-2 softmax weights
                    d12 = rt_pool.tile([P, 2], f32, tag="d12")
                    nc.vector.tensor_sub(d12[:, 0:1], m8[:, 0:1], m8[:, 1:2])
                    nc.vector.tensor_sub(d12[:, 1:2], m8[:, 1:2], m8[:, 0:1])
                    wt = rt_pool.tile([P, 2], f32, tag="wt")
                    nc.scalar.activation(wt[:], d12[:], AF.Sigmoid)
                    # combine weight for this core's expert (col E == own col m)
                    sel = rt_pool.tile([P, 2], f32, tag="sel")
                    nc.vector.tensor_tensor(
                        out=sel[:, 0:1], in0=S_[:, E:E + 1], in1=m8[:, 0:1],
                        op=ALU.is_equal,
                    )
                    nc.vector.tensor_tensor(
                        out=sel[:, 1:2], in0=S_[:, E:E + 1], in1=m8[:, 1:2],
                        op=ALU.is_equal,
                    )
                    wsel = rt_pool.tile([P, 2], f32, tag="wsel")
                    nc.vector.tensor_mul(wsel[:], sel[:], wt[:])
                    cw_t = cw_pool.tile([P, 1], f32, tag="cw")
                    nc.vector.tensor_add(cw_t[:], wsel[:, 0:1], wsel[:, 1:2])
                    cw.append(cw_t)
                    # aux-loss softmax accumulation
                    ex = rt_pool.tile([P, E], f32, tag="ex")
                    nc.scalar.activation(ex[:], S_[:, 0:E], AF.Exp)
                    rs = rt_pool.tile([P, 1], f32, tag="rs")
                    nc.vector.reduce_sum(out=rs[:], in_=ex[:], axis=AX.X)
                    rp = rt_pool.tile([P, 1], f32, tag="rp")
                    nc.vector.reciprocal(rp[:], rs[:])
                    pr = rt_pool.tile([P, E], f32, tag="pr")
                    nc.vector.tensor_scalar(
                        out=pr[:], in0=ex[:], scalar1=rp[:], scalar2=None,
                        op0=ALU.mult,
                    )
                    nc.vector.tensor_add(usage[:], usage[:], pr[:])

                # ---- experts: two h-halves; second half adds into yacc ----
                yacc = {}
                for hh in range(NHH):
                    # SwiGLU: g = silu(x@w1) * (x@w3), laid out [h, t]
                    g = []
                    for hci in range(HCH):
                        hc = hh * HCH + hci
                        w1c = w13_pool.tile([P, D], fmm, tag="w1c")
                        load(w1c[:], w1p[hc])
                        w3c = w13_pool.tile([P, D], fmm, tag="w3c")
                        load(w3c[:], w3p[hc])
                        g_hc = g_pool.tile([P, GT], fmm, tag="g")
                        for sub in range(NSUB):
                            ts = slice(sub * SUB, (sub + 1) * SUB)
                            ph1 = ps_h.tile([P, SUB], f32, tag="ph")
                            for dc in range(NDC):
                                nc.tensor.matmul(
                                    out=ph1[:],
                                    lhsT=w1c[:, dc * P:(dc + 1) * P],
                                    rhs=xt[dc][:, ts],
                                    start=(dc == 0),
                                    stop=(dc == NDC - 1),
                                )
                            ph3 = ps_h.tile([P, SUB], f32, tag="ph")
                            for dc in range(NDC):
                                nc.tensor.matmul(
                                    out=ph3[:],
                                    lhsT=w3c[:, dc * P:(dc + 1) * P],
                                    rhs=xt[dc][:, ts],
                                    start=(dc == 0),
                                    stop=(dc == NDC - 1),
                                )
                            g1 = gtmp_pool.tile([P, SUB], f32, tag="g1")
                            if use_silu:
                                nc.scalar.activation(g1[:], ph1[:], AF.Silu)
                                nc.vector.tensor_mul(g_hc[:, ts], g1[:], ph3[:])
                            else:
                                # CoreSim lacks Silu; silu(x) = x * sigmoid(x)
                                nc.scalar.activation(g1[:], ph1[:], AF.Sigmoid)
                                g2 = gtmp_pool.tile([P, SUB], f32, tag="g2")
                                nc.vector.tensor_mul(g2[:], g1[:], ph1[:])
                                nc.vector.tensor_mul(g_hc[:, ts], g2[:], ph3[:])
                        g.append(g_hc)

                    # partial y for this h-half: (g^T @ w2_half)
                    for dh in range(NDH):
                        ds_ = slice(dh * SUB, (dh + 1) * SUB)
                        w2c = []
                        for hci in range(HCH):
                            hc = hh * HCH + hci
                            t_ = w2_pool.tile([P, SUB], fmm, tag="w2c")
                            load(t_[:], w2[hc * P:(hc + 1) * P, ds_])
                            w2c.append(t_)
                        for tt in range(NTT):
                            po = ps_o.tile([P, SUB], f32, tag="po")
                            for hci in range(HCH):
                                nc.tensor.matmul(
                                    out=po[:],
                                    lhsT=g[hci][:, tt * P:(tt + 1) * P],
                                    rhs=w2c[hci][:],
                                    start=(hci == 0),
                                    stop=(hci == HCH - 1),
                                )
                            if hh == 0:
                                ya = yac_pool.tile([P, SUB], f32, tag="ya")
                                nc.scalar.activation(ya[:], po[:], AF.Copy)
                                yacc[(tt, dh)] = ya
                            else:
                                ya = yacc[(tt, dh)]
                                nc.vector.tensor_add(ya[:], ya[:], po[:])
                                ysb = y_pool.tile([P, SUB], f32, tag="ysb")
                                nc.scalar.activation(
                                    ysb[:], ya[:], AF.Copy, scale=cw[tt][:],
                                )
                                nc.sync.dma_start(
                                    out=y[b * GT + tt * P: b * GT + (tt + 1) * P,
                                          dh * SUB:(dh + 1) * SUB],
                                    in_=ysb[:],
                                )

            # ---- aux loss epilogue ----
            pu = ps_s.tile([1, E], f32, tag="ps")
            nc.tensor.matmul(
                out=pu[:], lhsT=ones_col[:], rhs=usage[:, 0:E],
                start=True, stop=True,
            )
            u_sb = rt_pool.tile([1, E], f32, tag="u")
            nc.vector.tensor_scalar(
                out=u_sb[:], in0=pu[:], scalar1=1.0 / T, scalar2=None,
                op0=ALU.mult,
            )
            sq = rt_pool.tile([1, E], f32, tag="sq")
            nc.vector.tensor_mul(sq[:], u_sb[:], u_sb[:])
            s1 = rt_pool.tile([1, 1], f32, tag="s1")
            nc.vector.reduce_sum(out=s1[:], in_=sq[:], axis=AX.X)
            nc.vector.tensor_scalar(
                out=s1[:], in0=s1[:], scalar1=float(E), scalar2=None,
                op0=ALU.mult,
            )
            nc.sync.dma_start(out=aux[:], in_=s1[:])

    nc.compile()
    return nc


def build_sparse(use_f32r=USE_F32R, use_silu=True):
    """Expert-parallel sparse kernel: on-device router over all T tokens,
    on-device compaction (prefix-sum permutation + indirect-DMA gather),
    SwiGLU over CAP tokens only, combine-weight-scaled compact output.
    Host scatter-adds y_compact rows at idx[:CAP] over the 8 cores."""
    import concourse.bacc as bacc
    import concourse.bass as bass
    import concourse.mybir as mybir
    import concourse.tile as tile
    from concourse.masks import make_identity

    dt = mybir.dt
    f32 = dt.float32
    i32 = dt.int32
    fmm = dt.float32r if use_f32r else f32
    AF = mybir.ActivationFunctionType
    ALU = mybir.AluOpType
    AX = mybir.AxisListType

    nc = bacc.Bacc("TRN2", target_bir_lowering=False, debug=False, num_devices=E)

    def load(out, in_):
        if out.dtype == dt.float32r:
            return nc.gpsimd.dma_start(out=out, in_=in_)
        return nc.sync.dma_start(out=out, in_=in_)

    x_d = nc.dram_tensor("x", [T, D], f32, kind="ExternalInput")
    xT = nc.dram_tensor("xT", [D, T], f32, kind="ExternalInput")
    gate_aug = nc.dram_tensor("gate_aug", [D, E + 1], f32, kind="ExternalInput")
    w1p = nc.dram_tensor("w1p", [NHC, P, D], f32, kind="ExternalInput")
    w3p = nc.dram_tensor("w3p", [NHC, P, D], f32, kind="ExternalInput")
    w2 = nc.dram_tensor("w2", [H, D], f32, kind="ExternalInput")
    yc = nc.dram_tensor("yc", [CAP, D], f32, kind="ExternalOutput")
    idx_out = nc.dram_tensor("idx", [T, 1], i32, kind="ExternalOutput")
    aux = nc.dram_tensor("aux", [1, 1], f32, kind="ExternalOutput")

    with tile.TileContext(nc) as tc:
        with (
            tc.tile_pool(name="g", bufs=HCH + 1) as g_pool,
            tc.tile_pool(name="xgT", bufs=1) as xgT_pool,
            tc.tile_pool(name="w13", bufs=2) as w13_pool,
            tc.tile_pool(name="w2p", bufs=HCH + 1) as w2_pool,
            tc.tile_pool(name="yac", bufs=8 * NDH + 1) as yac_pool,
            tc.tile_pool(name="small", bufs=1) as small_pool,
            tc.tile_pool(name="rt", bufs=3) as rt_pool,
            tc.tile_pool(name="perm", bufs=1) as perm_pool,
            tc.tile_pool(name="cwc", bufs=CAP // P + 1) as cwc_pool,
            tc.tile_pool(name="idxp", bufs=4) as idx_pool,
            tc.tile_pool(name="ysb", bufs=3) as y_pool,
            tc.tile_pool(name="gtmp", bufs=2) as gtmp_pool,
            tc.tile_pool(name="dints", bufs=1, space="DRAM") as dram_pool,
            tc.tile_pool(name="ps_s", bufs=2, space="PSUM") as ps_s,
            tc.tile_pool(name="ps_h", bufs=4, space="PSUM") as ps_h,
            tc.tile_pool(name="ps_o", bufs=2, space="PSUM") as ps_o,
        ):
            # ---- constants / persistent ----
            gate_sb = small_pool.tile([P, NDC, E + 1], f32, tag="gate")
            for dc in range(NDC):
                nc.sync.dma_start(
                    out=gate_sb[:, dc, :], in_=gate_aug[dc * P:(dc + 1) * P, :]
                )
            usage = small_pool.tile([P, E], f32, tag="usage")
            nc.vector.memset(usage[:], 0.0)
            ones_col = small_pool.tile([P, 1], f32, tag="ones")
            nc.vector.memset(ones_col[:], 1.0)
            ones_row = small_pool.tile([1, P], f32, tag="onesr")
            nc.vector.memset(ones_row[:], 1.0)
            ident = small_pool.tile([P, P], f32, tag="ident")
            make_identity(nc, ident[:])
            # strict-upper masks: su[q, c] = 1 iff q < c
            su128 = small_pool.tile([P, P], f32, tag="su128")
            nc.gpsimd.memset(su128[:], 0.0)
            nc.gpsimd.affine_select(
                out=su128[:], in_=su128[:], compare_op=ALU.is_ge, fill=1.0,
                base=0, pattern=[[-1, P]], channel_multiplier=1,
            )
            su64 = small_pool.tile([NJT, NJT], f32, tag="su64")
            nc.gpsimd.memset(su64[:], 0.0)
            nc.gpsimd.affine_select(
                out=su64[:], in_=su64[:], compare_op=ALU.is_ge, fill=1.0,
                base=0, pattern=[[-1, NJT]], channel_multiplier=1,
            )
            sel_all = perm_pool.tile([P, NJT], f32, tag="sel_all")
            # 8 scatter destinations: disjoint-slot writes to one tensor
            # would WAW-serialize (~9us per indirect call); round-robin
            # across 8 tiles keeps the SWDGE queue busy, merged at gather.
            NSC = 8
            idx_r = [
                dram_pool.tile([T, 1], i32, tag=f"idx_d{r}", name=f"idx_d{r}")
                for r in range(NSC)
            ]
            cw_dram = dram_pool.tile([T, 1], f32, tag="cw_d")
            neg1 = perm_pool.tile([P, NJT], i32, tag="neg1")
            nc.vector.memset(neg1[:], -1)
            for r in range(NSC):
                nc.sync.dma_start(
                    out=idx_r[r][:].rearrange("(a b) c -> a (b c)", a=P),
                    in_=neg1[:],
                )

            # ---- phase R: router over all tokens ----
            for jg in range(NJT // 4):
                xr = []
                for dc in range(NDC):
                    t_ = g_pool.tile([P, 4 * P], f32, tag="g", name=f"xr{jg}_{dc}")
                    nc.sync.dma_start(
                        out=t_[:],
                        in_=xT[dc * P:(dc + 1) * P,
                               jg * 4 * P:(jg + 1) * 4 * P],
                    )
                    xr.append(t_)
                for jj in range(4):
                    j = jg * 4 + jj
                    ps = ps_s.tile([P, E + 1], f32, tag="ps")
                    for dc in range(NDC):
                        nc.tensor.matmul(
                            out=ps[:],
                            lhsT=xr[dc][:, jj * P:(jj + 1) * P],
                            rhs=gate_sb[:, dc, :],
                            start=(dc == 0),
                            stop=(dc == NDC - 1),
                        )
                    S_ = rt_pool.tile([P, E + 1], f32, tag="S")
                    nc.vector.tensor_copy(S_[:], ps[:])
                    m8 = rt_pool.tile([P, 8], f32, tag="m8")
                    nc.vector.max(m8[:], S_[:, 0:E])
                    d12 = rt_pool.tile([P, 2], f32, tag="d12")
                    nc.vector.tensor_sub(d12[:, 0:1], m8[:, 0:1], m8[:, 1:2])
                    nc.vector.tensor_sub(d12[:, 1:2], m8[:, 1:2], m8[:, 0:1])
                    wt = rt_pool.tile([P, 2], f32, tag="wt")
                    nc.scalar.activation(wt[:], d12[:], AF.Sigmoid)
                    sel = rt_pool.tile([P, 2], f32, tag="sel")
                    nc.vector.tensor_tensor(
                        out=sel[:, 0:1], in0=S_[:, E:E + 1], in1=m8[:, 0:1],
                        op=ALU.is_equal,
                    )
                    nc.vector.tensor_tensor(
                        out=sel[:, 1:2], in0=S_[:, E:E + 1], in1=m8[:, 1:2],
                        op=ALU.is_equal,
                    )
                    wsel = rt_pool.tile([P, 2], f32, tag="wsel")
                    nc.vector.tensor_mul(wsel[:], sel[:], wt[:])
                    cw_t = rt_pool.tile([P, 1], f32, tag="cw")
                    nc.vector.tensor_add(cw_t[:], wsel[:, 0:1], wsel[:, 1:2])
                    nc.sync.dma_start(
                        out=cw_dram[j * P:(j + 1) * P, :], in_=cw_t[:]
                    )
                    nc.vector.tensor_tensor(
                        out=sel_all[:, j:j + 1], in0=sel[:, 0:1],
                        in1=sel[:, 1:2], op=ALU.max,
                    )
                    # aux-loss softmax accumulation
                    ex = rt_pool.tile([P, E], f32, tag="ex")
                    nc.scalar.activation(ex[:], S_[:, 0:E], AF.Exp)
                    rs = rt_pool.tile([P, 1], f32, tag="rs")
                    nc.vector.reduce_sum(out=rs[:], in_=ex[:], axis=AX.X)
                    rp = rt_pool.tile([P, 1], f32, tag="rp")
                    nc.vector.reciprocal(rp[:], rs[:])
                    pr = rt_pool.tile([P, E], f32, tag="pr")
                    nc.vector.tensor_scalar(
                        out=pr[:], in0=ex[:], scalar1=rp[:], scalar2=None,
                        op0=ALU.mult,
                    )
                    nc.vector.tensor_add(usage[:], usage[:], pr[:])

            # ---- phase P: compact-slot permutation ----
            seln_all = perm_pool.tile([P, NJT], f32, tag="seln")
            nc.vector.tensor_scalar(
                out=seln_all[:], in0=sel_all[:], scalar1=0.0, scalar2=None,
                op0=ALU.is_equal,
            )

            def prefix(mat):  # exclusive prefix slots for 0/1 matrix [P, NJT]
                pfp = ps_s.tile([P, NJT], f32, tag="ps", name=f"pf_{mat.tensor.name}")
                nc.tensor.matmul(out=pfp[:], lhsT=su128[:], rhs=mat[:],
                                 start=True, stop=True)
                pf_sb = perm_pool.tile([P, NJT], f32, tag=f"pf_{mat.tensor.name}")
                nc.scalar.activation(pf_sb[:], pfp[:], AF.Copy)
                cntp = ps_s.tile([NJT, 1], f32, tag="ps", name=f"cnt_{mat.tensor.name}")
                nc.tensor.matmul(out=cntp[:], lhsT=mat[:], rhs=ones_col[:],
                                 start=True, stop=True)
                cnt_sb = perm_pool.tile([NJT, 1], f32, tag=f"cnt_{mat.tensor.name}")
                nc.scalar.activation(cnt_sb[:], cntp[:], AF.Copy)
                basep = ps_s.tile([1, NJT], f32, tag="ps", name=f"base_{mat.tensor.name}")
                nc.tensor.matmul(out=basep[:], lhsT=cnt_sb[:], rhs=su64[:],
                                 start=True, stop=True)
                base_sb = perm_pool.tile([1, NJT], f32, tag=f"base_{mat.tensor.name}")
                nc.scalar.activation(base_sb[:], basep[:], AF.Copy)
                bcp = ps_s.tile([P, NJT], f32, tag="ps", name=f"bc_{mat.tensor.name}")
                nc.tensor.matmul(out=bcp[:], lhsT=ones_row[:], rhs=base_sb[:],
                                 start=True, stop=True)
                slot = perm_pool.tile([P, NJT], f32, tag=f"slot_{mat.tensor.name}")
                nc.vector.tensor_add(slot[:], pf_sb[:], bcp[:])
                return slot

            slot_s = prefix(sel_all)
            slot_n = prefix(seln_all)
            # not-selected tokens fill slots T-1, T-2, ... (trash region)
            nc.vector.tensor_scalar(
                out=slot_n[:], in0=slot_n[:], scalar1=-1.0,
                scalar2=float(T - 1), op0=ALU.mult, op1=ALU.add,
            )
            slot_f = perm_pool.tile([P, NJT], f32, tag="slot_f")
            nc.vector.tensor_mul(slot_f[:], seln_all[:], slot_n[:])
            ssel = perm_pool.tile([P, NJT], f32, tag="ssel")
            nc.vector.tensor_mul(ssel[:], sel_all[:], slot_s[:])
            nc.vector.tensor_add(slot_f[:], slot_f[:], ssel[:])
            slot_i = perm_pool.tile([P, NJT], i32, tag="slot_i")
            nc.vector.tensor_copy(slot_i[:], slot_f[:])
            iota_t = perm_pool.tile([P, NJT], i32, tag="iota")
            nc.gpsimd.iota(iota_t[:], pattern=[[P, NJT]], base=0,
                           channel_multiplier=1)
            # NB: multi-column offset APs silently misbehave on HW DGE;
            # scatter one [128, 1] column of token ids per call.
            for j in range(NJT):
                nc.gpsimd.indirect_dma_start(
                    out=idx_r[j % NSC][:],
                    out_offset=bass.IndirectOffsetOnAxis(
                        ap=slot_i[:, j:j + 1], axis=0),
                    in_=iota_t[:, j:j + 1], in_offset=None,
                )

            # ---- phase G+M: gather + SwiGLU + combine per compact block ----
            cwc = {}
            for (t0, gt) in CBLKS:
                ntt = gt // P
                nsub = max(1, gt // SUB)
                subs = [
                    (s * SUB, min(SUB, gt - s * SUB)) for s in range(nsub)
                ]
                xgT = []
                for dc in range(NDC):
                    t_ = xgT_pool.tile([P, gt], fmm, tag=f"xgT{dc}",
                                       name=f"xgT{t0}_{dc}")
                    xgT.append(t_)
                for k in range(ntt):
                    ks = t0 + k * P
                    # merge the 8 scatter shards: exactly one is != -1
                    cand = idx_pool.tile([P, NSC], i32, tag="cand")
                    for r in range(NSC):
                        nc.sync.dma_start(
                            out=cand[:, r:r + 1], in_=idx_r[r][ks:ks + P, :]
                        )
                    idx_k = idx_pool.tile([P, 1], i32, tag="idxk")
                    nc.vector.tensor_tensor(
                        out=idx_k[:], in0=cand[:, 0:1], in1=cand[:, 1:2],
                        op=ALU.max,
                    )
                    for r in range(2, NSC):
                        nc.vector.tensor_tensor(
                            out=idx_k[:], in0=idx_k[:], in1=cand[:, r:r + 1],
                            op=ALU.max,
                        )
                    nc.sync.dma_start(out=idx_out[ks:ks + P, :], in_=idx_k[:])
                    xg_k = g_pool.tile([P, D], f32, tag="g", name=f"xg{ks}")
                    nc.gpsimd.indirect_dma_start(
                        out=xg_k[:], out_offset=None, in_=x_d[:],
                        in_offset=bass.IndirectOffsetOnAxis(
                            ap=idx_k[:, :1], axis=0),
                    )
                    cw_k = cwc_pool.tile([P, 1], f32, tag="cwc",
                                         name=f"cwk{ks}")
                    nc.gpsimd.indirect_dma_start(
                        out=cw_k[:], out_offset=None, in_=cw_dram[:],
                        in_offset=bass.IndirectOffsetOnAxis(
                            ap=idx_k[:, :1], axis=0),
                    )
                    cwc[ks // P] = cw_k
                    for dc in range(NDC):
                        tp = ps_s.tile([P, P], f32, tag="ps",
                                       name=f"tp{ks}_{dc}")
                        nc.tensor.transpose(
                            tp[:], xg_k[:, dc * P:(dc + 1) * P], ident[:]
                        )
                        nc.vector.tensor_copy(
                            xgT[dc][:, k * P:(k + 1) * P], tp[:]
                        )

                def swiglu_hc(hc, gt, t0, subs, xgT):
                    w1c = w13_pool.tile([P, D], fmm, tag="w1c")
                    load(w1c[:], w1p[hc])
                    w3c = w13_pool.tile([P, D], fmm, tag="w3c")
                    load(w3c[:], w3p[hc])
                    g_hc = g_pool.tile([P, gt], fmm, tag="g",
                                       name=f"g{t0}_{hc}")
                    for (s0, sl) in subs:
                        ts = slice(s0, s0 + sl)
                        ph1 = ps_h.tile([P, sl], f32, tag="ph")
                        for dc in range(NDC):
                            nc.tensor.matmul(
                                out=ph1[:],
                                lhsT=w1c[:, dc * P:(dc + 1) * P],
                                rhs=xgT[dc][:, ts],
                                start=(dc == 0), stop=(dc == NDC - 1),
                            )
                        ph3 = ps_h.tile([P, sl], f32, tag="ph")
                        for dc in range(NDC):
                            nc.tensor.matmul(
                                out=ph3[:],
                                lhsT=w3c[:, dc * P:(dc + 1) * P],
                                rhs=xgT[dc][:, ts],
                                start=(dc == 0), stop=(dc == NDC - 1),
                            )
                        g1 = gtmp_pool.tile([P, sl], f32, tag="g1")
                        if use_silu:
                            nc.scalar.activation(g1[:], ph1[:], AF.Silu)
                            nc.vector.tensor_mul(g_hc[:, ts], g1[:], ph3[:])
                        else:
                            nc.scalar.activation(g1[:], ph1[:], AF.Sigmoid)
                            g2 = gtmp_pool.tile([P, sl], f32, tag="g2")
                            nc.vector.tensor_mul(g2[:], g1[:], ph1[:])
                            nc.vector.tensor_mul(g_hc[:, ts], g2[:], ph3[:])
                    return g_hc

                def mm3_group(hh, dh, g, t0, ntt, yacc):
                    ds_ = slice(dh * SUB, (dh + 1) * SUB)
                    w2c = []
                    for hci in range(HCH):
                        hc = hh * HCH + hci
                        t_ = w2_pool.tile([P, SUB], fmm, tag="w2c")
                        load(t_[:], w2[hc * P:(hc + 1) * P, ds_])
                        w2c.append(t_)
                    for tt in range(ntt):
                        po = ps_o.tile([P, SUB], f32, tag="po")
                        for hci in range(HCH):
                            nc.tensor.matmul(
                                out=po[:],
                                lhsT=g[hci][:, tt * P:(tt + 1) * P],
                                rhs=w2c[hci][:],
                                start=(hci == 0), stop=(hci == HCH - 1),
                            )
                        if hh == 0:
                            ya = yac_pool.tile([P, SUB], f32, tag="ya")
                            nc.scalar.activation(ya[:], po[:], AF.Copy)
                            yacc[(tt, dh)] = ya
                        else:
                            ya = yacc[(tt, dh)]
                            nc.vector.tensor_add(ya[:], ya[:], po[:])
                            ysb = y_pool.tile([P, SUB], f32, tag="ysb")
                            nc.scalar.activation(
                                ysb[:], ya[:], AF.Copy,
                                scale=cwc[(t0 + tt * P) // P][:],
                            )
                            nc.sync.dma_start(
                                out=yc[t0 + tt * P: t0 + (tt + 1) * P,
                                       dh * SUB:(dh + 1) * SUB],
                                in_=ysb[:],
                            )

                yacc = {}
                for hh in range(NHH):
                    g = [
                        swiglu_hc(hh * HCH + hci, gt, t0, subs, xgT)
                        for hci in range(HCH)
                    ]
                    for dh in range(NDH):
                        mm3_group(hh, dh, g, t0, ntt, yacc)

            # ---- aux loss epilogue ----
            pu = ps_s.tile([1, E], f32, tag="ps")
            nc.tensor.matmul(
                out=pu[:], lhsT=ones_col[:], rhs=usage[:, 0:E],
                start=True, stop=True,
            )
            u_sb = rt_pool.tile([1, E], f32, tag="u")
            nc.vector.tensor_scalar(
                out=u_sb[:], in0=pu[:], scalar1=1.0 / T, scalar2=None,
                op0=ALU.mult,
            )
            sq = rt_pool.tile([1, E], f32, tag="sq")
            nc.vector.tensor_mul(sq[:], u_sb[:], u_sb[:])
            s1 = rt_pool.tile([1, 1], f32, tag="s1")
            nc.vector.reduce_sum(out=s1[:], in_=sq[:], axis=AX.X)
            nc.vector.tensor_scalar(
                out=s1[:], in0=s1[:], scalar1=float(E), scalar2=None,
                op0=ALU.mult,
            )
            nc.sync.dma_start(out=aux[:], in_=s1[:])

    nc.compile()
    return nc


def _prep_inputs(x, gate_w, w1, w2, w3):
    xT = np.ascontiguousarray(x.reshape(T, D).T)
    in_maps = []
    for m in range(E):
        gate_aug = np.concatenate([gate_w, gate_w[:, m:m + 1]], axis=1)
        gate_aug = np.ascontiguousarray(gate_aug)
        w1p = np.ascontiguousarray(
            w1[m].reshape(NDC, P, NHC, P).transpose(2, 1, 0, 3).reshape(NHC, P, D)
        )
        w3p = np.ascontiguousarray(
            w3[m].reshape(NDC, P, NHC, P).transpose(2, 1, 0, 3).reshape(NHC, P, D)
        )
        in_maps.append({
            "xT": xT,
            "gate_aug": gate_aug,
            "w1p": w1p,
            "w3p": w3p,
            "w2": np.ascontiguousarray(w2[m]),
        })
    return in_maps


_CACHE = {}


def _get_program(sparse):
    key = ("sparse" if sparse else "dense", USE_F32R)
    if key not in _CACHE:
        _CACHE[key] = (
            build_sparse(USE_F32R) if sparse else build_program(USE_F32R)
        )
    return _CACHE[key]


def _sparse_ok(x, gate_w):
    """Capacity check: sparse path drops tokens if any expert exceeds CAP."""
    S_ = x.reshape(T, D) @ gate_w
    top2 = np.argpartition(-S_, 2, axis=1)[:, :2]
    counts = np.bincount(top2.ravel(), minlength=E)
    return counts.max() <= CAP - 64


def run_on_cores(x, gate_w, w1, w2, w3, trace=False, sparse=None):
    from concourse.bass_utils import run_bass_kernel_spmd

    if sparse is None:
        sparse = USE_SPARSE and _sparse_ok(x, gate_w)
    nc = _get_program(sparse)
    in_maps = _prep_inputs(x, gate_w, w1, w2, w3)
    if sparse:
        x2d = np.ascontiguousarray(x.reshape(T, D))
        for m_ in in_maps:
            m_["x"] = x2d
    res = run_bass_kernel_spmd(
        nc, in_maps, core_ids=list(range(E)), trace=trace,
    )
    return res, sparse


def kernel(x, gate_w, w1, w2, w3):
    x = np.asarray(x, dtype=np.float32)
    gate_w = np.asarray(gate_w, dtype=np.float32)
    w1 = np.asarray(w1, dtype=np.float32)
    w2 = np.asarray(w2, dtype=np.float32)
    w3 = np.asarray(w3, dtype=np.float32)

    res, sparse = run_on_cores(x, gate_w, w1, w2, w3, trace=False)
    out = np.zeros((T, D), dtype=np.float32)
    if sparse:
        for m in range(E):
            idx = res.results[m]["idx"].ravel()[:CAP]
            out[idx] += res.results[m]["yc"]
    else:
        for m in range(E):
            out += res.results[m]["y"]
    aux = np.float32(res.results[0]["aux"].reshape(())[()])
    return out.reshape(B, S, D), aux
